# revision 45
# baseline (speedup 1.0000x reference)
"""Trainium2 Bass kernel for nn_AutoDecoder (moe_routing).

Reference computation (per full input):
  x: [S=3072, B=32, C=512]; rows s%3==1 are "brick" tokens, s%3==2 are
  "combined" tokens (s%3==0 PAD rows are dead). For each (timestep, batch)
  pair:
    brick:  logits[0:80]    = x_brick @ [Ws|Wc]            (+ biases)
    comb:   h = relu(relu(x_comb @ W1 + b1) @ W2 + b2)
            logits[80:1000] = h @ Wh + bh
  out: [TS=1024, B=32, A=1000]

Strategy: data-parallel over batch (4 batch entries per core, 8 cores),
weights replicated. The host pre-packs x into feature-major fp16 tiles
(one [C, ntok] panel per readout name per core), so the device does no
transposes at all: per 512-token block it streams the 2-layer MLP
(fp16 weights, fp32 PSUM accumulation, relu+bias on ACT) and then the
head matmuls with the activations as stationary operands, producing
token-major logits in PSUM. The DVE adds the output bias while
downcasting to fp16, and fully contiguous DMAs write the fp16 logits
back (host upcasts to fp32). Heads for block i are emitted during
block i+1 so the PE never waits on ACT output of the same block.
"""
import sys

if "/opt/trn_rl_repo" not in sys.path:
    sys.path.append("/opt/trn_rl_repo")

import numpy as np

import concourse.bass as bass
from concourse import bacc
import concourse.mybir as mybir
import concourse.tile as tile
from concourse.bass import ts
from concourse.bass_utils import run_bass_kernel_spmd

F32 = mybir.dt.float32
F16 = mybir.dt.float16
RELU = mybir.ActivationFunctionType.Relu

# problem dims (hardcoded; kernel.py must be self-contained)
S, B, C = 3072, 32, 512
TS_ = S // 3                    # 1024 timesteps
NUM_SHAPES, NUM_COLORS, N_COMBINED = 64, 16, 920
NBRICK = NUM_SHAPES + NUM_COLORS  # 80
A = NBRICK + N_COMBINED           # 1000
NCORES = 8
BL = B // NCORES                  # 4 batch entries per core
NTOKC = TS_ * BL                  # 4096 tokens per name per core
TT = 128                          # tokens per tok-tile
TPB = TT // BL                    # 32 timesteps per tok-tile
KC = C // 128                     # 4 contraction chunks

_BUILD_CACHE = {}


def _build():
    if "nc" in _BUILD_CACHE:
        return _BUILD_CACHE["nc"]
    nc = bacc.Bacc("TRN2", target_bir_lowering=False, debug=False)

    # feature-major fp16 x: rows (ni*KC + k)*128 + p  <->  name ni, feature
    # k*128+p; cols are tokens t*BL+b
    xt_d = nc.declare_dram_parameter("xt", [2 * KC * 128, NTOKC], F16, isOutput=False)
    w1_d = nc.declare_dram_parameter("w1", [C, C], F16, isOutput=False)
    w2_d = nc.declare_dram_parameter("w2", [C, C], F16, isOutput=False)
    wh_d = nc.declare_dram_parameter("wh", [C, N_COMBINED], F16, isOutput=False)
    wsc_d = nc.declare_dram_parameter("wsc", [C, NBRICK], F16, isOutput=False)
    b12_d = nc.declare_dram_parameter("b12", [128, 2 * KC], F32, isOutput=False)
    bA_d = nc.declare_dram_parameter("biasA", [128, A], F32, isOutput=False)
    out_d = nc.declare_dram_parameter("out", [TS_ * BL, A], F16, isOutput=True)

    with tile.TileContext(nc) as tc:
        with (
            tc.tile_pool(name="const", bufs=1) as const,
            tc.tile_pool(name="xt", bufs=4) as xt_p,
            tc.tile_pool(name="h", bufs=4) as h_p,
            tc.tile_pool(name="osb", bufs=4) as o_p,
            tc.tile_pool(name="psh", bufs=4, space=bass.MemorySpace.PSUM) as ps_h,
            tc.tile_pool(name="psc", bufs=2, space=bass.MemorySpace.PSUM) as ps_c,
        ):
            # HAM warmup: a short burst of dummy matmuls at t=0 (on a memset
            # scratch, no DMA dependency) so the PE clock ramp is already in
            # progress while the first weight/x DMAs land. Kept minimal —
            # real matmuls during the ramp make progress, dummies don't.
            warm_src = const.tile([128, 128], F16, tag="warm")
            nc.vector.memset(warm_src[:], 0.0)
            warm = ps_h.tile([128, 512], F32, tag="hps")
            for _ in range(26):
                nc.tensor.matmul(warm[0:64, 0:64], warm_src[:, 0:64], warm_src[:, 0:64])
            # pre-fire the one-time ACT activation-table load so the first
            # real relu doesn't pay ~1.3us for it
            warm_act = const.tile([128, 1], F32, tag="warmact")
            nc.scalar.activation(warm_act[0:1, 0:1], warm_src[0:1, 0:1], RELU)

            # Startup DMA placement (per-HW-queue ~115 GB/s for 1KB-line
            # tiles; tiny-line tensors are descriptor-bound; SW-DGE gpsimd
            # queue coalesces into 4KB packets). Per-k-chunk tiles so the
            # k-outer block-0 compute streams with chunk arrivals (the tile
            # dependency is whole-DMA completion):
            #   sync:   w1 k0..k3, w2 k0..k3, biasA, then out-lo halves
            #   scalar: block-0 comb x, wh, then out-hi halves
            #   gpsimd: b1/b2 (tiny), wsc, block-0 brick x, then per-block
            #           x prefetch only
            w1_sb = []
            w2_sb = []
            for name, dram, out_list in (("w1", w1_d, w1_sb), ("w2", w2_d, w2_sb)):
                for k in range(KC):
                    t = const.tile([128, C], F16, tag=f"{name}_{k}")
                    out_list.append(t)
            for wsb, dram in ((w1_sb, w1_d), (w2_sb, w2_d)):
                for k in range(KC):
                    nc.sync.dma_start(wsb[k][:], dram[ts(k, 128), :])
            b12_sb = const.tile([128, 2 * KC], F32, tag="b12")
            nc.gpsimd.dma_start(b12_sb[:], b12_d[:, :])
            wh_sb = []
            wsc_sb = []
            for k in range(KC):
                t = const.tile([128, NBRICK], F16, tag=f"wsc_{k}")
                nc.gpsimd.dma_start(t[:], wsc_d[ts(k, 128), :])
                wsc_sb.append(t)

            def w1s(k, m):
                return w1_sb[k][:, ts(m, 128)]

            def w2s(k, m):
                return w2_sb[k][:, ts(m, 128)]

            def load_late_consts():
                # wh on scalar behind block-0 comb x; biasA last on sync
                for k in range(KC):
                    t = const.tile([128, N_COMBINED], F16, tag=f"wh_{k}")
                    nc.scalar.dma_start(t[:], wh_d[ts(k, 128), :])
                    wh_sb.append(t)
                bA_sb = const.tile([128, A], F32, tag="biasA")
                nc.sync.dma_start(bA_sb[:], bA_d[:, :])
                return bA_sb

            # heads for block i, emitted during block i+1: token-major
            # logits straight into PSUM ([0:80] brick, [80:1000] comb),
            # DVE bias-add + fp16 downcast, contiguous DMA out.
            def finals(pb):
                for t in range(pb["nt"]):
                    pco = ps_c.tile([128, 1024], F32, tag="combo")
                    for k in range(KC):
                        lhs = pb["h2"][k][:, ts(t, 128)]
                        nc.tensor.matmul(
                            pco[:, NBRICK:512],
                            lhs,
                            wh_sb[k][:, 0 : 512 - NBRICK],
                            start=(k == 0),
                            stop=(k == KC - 1),
                        )
                        nc.tensor.matmul(
                            pco[:, 512:A],
                            lhs,
                            wh_sb[k][:, 512 - NBRICK : N_COMBINED],
                            start=(k == 0),
                            stop=(k == KC - 1),
                        )
                    for k in range(KC):
                        nc.tensor.matmul(
                            pco[:, 0:NBRICK],
                            pb["xtb"][k][:, ts(t, 128)],
                            wsc_sb[k][:],
                            start=(k == 0),
                            stop=(k == KC - 1),
                        )
                    # bias-add + fp16 downcast in two column halves so the
                    # first half's DMA fires while the DVE does the second
                    # half; halves go to different queues (the last block's
                    # upper half uses the by-then-idle scalar HW queue so
                    # the drain tail is short)
                    ot = o_p.tile([128, A], F16, tag="osb")
                    r0 = (pb["t0"] + t * TPB) * BL
                    nc.vector.tensor_add(ot[:, 0:512], pco[:, 0:512], bA_sb[:, 0:512])
                    nc.sync.dma_start(out_d[r0 : r0 + TT, 0:512], ot[:, 0:512])
                    nc.vector.tensor_add(ot[:, 512:A], pco[:, 512:A], bA_sb[:, 512:A])
                    nc.scalar.dma_start(out_d[r0 : r0 + TT, 512:A], ot[:, 512:A])

            # ---- main loop over blocks of tok-tiles (128 tokens each) ----
            # ramp-up/ramp-down schedule: small blocks first (fast pipeline
            # fill, early HAM release) and last (short drain tail).
            sched = [4] * 7 + [2, 1, 1]
            assert sum(sched) == TS_ // TPB
            offs = [sum(sched[:i]) for i in range(len(sched))]
            # heads deferral: the first ~25us are DMA-limited (weights still
            # streaming in at ~60-80 GB/s per queue), so the first blocks'
            # heads — which need wh — are pushed back 1-2 extra blocks; the
            # PE fills that time with MLP work that only needs w1/w2/x. The
            # backlog is worked off two-per-block once wh has landed.
            nblk = len(sched)
            flush_n = {2: 1, 3: 2}
            flush_n.update({i: 1 for i in range(4, nblk)})

            # feature-major fp16 x loads, issued one block AHEAD so the
            # queue has a full block period to deliver. Block 0's comb x
            # goes on the scalar HW queue (parallel with w1 on sync) and
            # its brick x on gpsimd; later blocks use the gpsimd
            # software-DGE queue entirely.
            def issue_name_loads(bi, ni, eng):
                nt = sched[bi]
                tok0 = offs[bi] * TT
                W_ = nt * TT
                out_list = []
                for k in range(KC):
                    tl = xt_p.tile([128, W_], F16, tag=f"xt{ni}{k}")
                    r0 = (ni * KC + k) * 128
                    eng.dma_start(tl[:], xt_d[r0 : r0 + 128, tok0 : tok0 + W_])
                    out_list.append(tl)
                return out_list

            def issue_loads(bi):
                return (
                    issue_name_loads(bi, 1, nc.gpsimd),
                    issue_name_loads(bi, 0, nc.gpsimd),
                )

            xtc0 = issue_name_loads(0, 1, nc.scalar)
            bA_sb = load_late_consts()
            issued = {1: issue_loads(1)}
            # block 0's brick x goes behind block 1's x on the SW queue —
            # it is not read until the deferred heads flush
            issued[0] = (xtc0, issue_name_loads(0, 0, nc.gpsimd))
            pendings = []
            for bi, nt in enumerate(sched):
                ti0 = offs[bi]
                t0 = ti0 * TPB
                W_ = nt * TT  # tokens per name in this block
                xtc, xtb = issued.pop(bi)
                if bi + 1 < len(sched) and bi + 1 not in issued:
                    issued[bi + 1] = issue_loads(bi + 1)

                # deferred heads of earlier blocks (see note above)
                for _ in range(flush_n.get(bi, 0)):
                    finals(pendings.pop(0))

                # comb MLP: hT[m] = relu(W[:,m-chunk].T @ inT + b). Block 0
                # runs k-outer so matmuls stream with the arriving weight/x
                # DMA chunks; later blocks run m-outer so the ACT engine
                # drains each m-psum while the next accumulates.
                def layer(ws, inp, btag, hname):
                    hs_list = []
                    if bi < 2:
                        phs = []
                        for _m in range(KC):
                            ph = ps_h.tile([128, W_], F32, tag="hps")
                            phs.append(ph)
                        for k in range(KC):
                            for m in range(KC):
                                nc.tensor.matmul(
                                    phs[m][:],
                                    ws(k, m),
                                    inp[k][:],
                                    start=(k == 0),
                                    stop=(k == KC - 1),
                                )
                        for m in range(KC):
                            hs = h_p.tile([128, W_], F16, tag=f"{hname}_{m}")
                            nc.scalar.activation(
                                hs[:],
                                phs[m][:],
                                RELU,
                                bias=b12_sb[:, btag + m : btag + m + 1],
                                scale=1.0,
                            )
                            hs_list.append(hs)
                    else:
                        for m in range(KC):
                            ph = ps_h.tile([128, W_], F32, tag="hps")
                            for k in range(KC):
                                nc.tensor.matmul(
                                    ph[:],
                                    ws(k, m),
                                    inp[k][:],
                                    start=(k == 0),
                                    stop=(k == KC - 1),
                                )
                            hs = h_p.tile([128, W_], F16, tag=f"{hname}_{m}")
                            nc.scalar.activation(
                                hs[:],
                                ph[:],
                                RELU,
                                bias=b12_sb[:, btag + m : btag + m + 1],
                                scale=1.0,
                            )
                            hs_list.append(hs)
                    return hs_list

                h1 = layer(w1s, xtc, 0, "h1")
                h2 = layer(w2s, h1, KC, "h2")

                pendings.append({"h2": h2, "xtb": xtb, "t0": t0, "nt": nt})
            assert len(pendings) == 1
            pendings[0]["last"] = True
            finals(pendings[0])

    nc.compile()
    _BUILD_CACHE["nc"] = nc
    return nc


def _prepare_inputs(inputs):
    """Host-side prep: validate/normalize routing, shard over batch,
    pre-pack x feature-major fp16, replicate weights. Returns in_maps
    for the 8 cores."""
    x = np.ascontiguousarray(np.asarray(inputs["x"], dtype=np.float32))
    readout_x = np.asarray(inputs["readout_x"], dtype=np.int32)
    W1 = np.asarray(inputs["W1"], dtype=np.float32)
    W2 = np.asarray(inputs["W2"], dtype=np.float32)
    Wh = np.asarray(inputs["Wh"], dtype=np.float32)
    Ws = np.asarray(inputs["Ws"], dtype=np.float32)
    Wc = np.asarray(inputs["Wc"], dtype=np.float32)
    b1 = np.asarray(inputs["b1"], dtype=np.float32)
    b2 = np.asarray(inputs["b2"], dtype=np.float32)
    bh = np.asarray(inputs["bh"], dtype=np.float32)
    bs = np.asarray(inputs["bs"], dtype=np.float32)
    bc = np.asarray(inputs["bc"], dtype=np.float32)

    # The kernel hardcodes the cyclic PAD/brick/comb routing. If the actual
    # readout pattern differs, permute x on the host so the device sees the
    # canonical layout (mirrors jnp.nonzero(..., size=ntok) semantics).
    ntok = TS_ * B
    rf = readout_x.reshape(-1)
    canonical = np.array_equal(
        readout_x, np.broadcast_to((np.arange(S, dtype=np.int32) % 3)[:, None], (S, B))
    )
    if not canonical:
        xf = x.reshape(S * B, C)
        xc = np.zeros_like(x).reshape(S * B, C)
        for name_idx in (1, 2):
            idx = np.nonzero(rf == name_idx)[0]
            if idx.shape[0] < ntok:
                idx = np.pad(idx, (0, ntok - idx.shape[0]))
            else:
                idx = idx[:ntok]
            tgt = (3 * (np.arange(ntok) // B) + name_idx) * B + (np.arange(ntok) % B)
            xc[tgt] = xf[idx]
        x = xc.reshape(S, B, C)

    # feature-major fp16 pack: [name, core, C, t, b] with brick first
    xh = x.astype(np.float16).reshape(TS_, 3, NCORES, BL, C)
    xp = xh[:, 1:3].transpose(1, 2, 4, 0, 3)  # [name, core, C, t, b]

    Wsc = np.ascontiguousarray(np.concatenate([Ws, Wc], axis=1).astype(np.float16))
    W1h = np.ascontiguousarray(W1.astype(np.float16))
    W2h = np.ascontiguousarray(W2.astype(np.float16))
    Whh = np.ascontiguousarray(Wh.astype(np.float16))
    b12 = np.ascontiguousarray(
        np.concatenate([b1.reshape(KC, 128).T, b2.reshape(KC, 128).T], axis=1)
    )
    biasA = np.concatenate([bs, bc, bh])
    biasA_b = np.ascontiguousarray(np.broadcast_to(biasA, (128, A)))

    in_maps = []
    for c in range(NCORES):
        # rows: (ni*KC + k)*128 + p = ni*512 + feature, brick ni=0 from
        # readout name 1, comb ni=1 from name 2
        xt_core = np.ascontiguousarray(
            xp[:, c].reshape(2 * C, NTOKC)
        )
        in_maps.append(
            {
                "xt": xt_core,
                "w1": W1h,
                "w2": W2h,
                "wh": Whh,
                "wsc": Wsc,
                "b12": b12,
                "biasA": biasA_b,
            }
        )
    return in_maps


def _run(inputs, trace=False, trace_kwargs=None):
    nc = _build()
    in_maps = _prepare_inputs(inputs)
    res = run_bass_kernel_spmd(
        nc,
        in_maps,
        list(range(NCORES)),
        trace=trace,
        **(trace_kwargs or {}),
    )
    out = np.empty((TS_, B, A), dtype=np.float32)
    for c in range(NCORES):
        out[:, c * BL : (c + 1) * BL, :] = (
            res.results[c]["out"].reshape(TS_, BL, A).astype(np.float32)
        )
    return out, res


def kernel(**inputs) -> np.ndarray:
    out, _ = _run(inputs, trace=False)
    return out


if __name__ == "__main__":
    nc = _build()
    print("built OK")


# revision 47
# speedup vs baseline: 1.0067x; 1.0067x over previous
"""Trainium2 Bass kernel for nn_AutoDecoder (moe_routing).

Reference computation (per full input):
  x: [S=3072, B=32, C=512]; rows s%3==1 are "brick" tokens, s%3==2 are
  "combined" tokens (s%3==0 PAD rows are dead). For each (timestep, batch)
  pair:
    brick:  logits[0:80]    = x_brick @ [Ws|Wc]            (+ biases)
    comb:   h = relu(relu(x_comb @ W1 + b1) @ W2 + b2)
            logits[80:1000] = h @ Wh + bh
  out: [TS=1024, B=32, A=1000]

Strategy: data-parallel over batch (4 batch entries per core, 8 cores),
weights replicated. The host pre-packs x into feature-major fp16 tiles
(one [C, ntok] panel per readout name per core), so the device does no
transposes at all: per 512-token block it streams the 2-layer MLP
(fp16 weights, fp32 PSUM accumulation, relu+bias on ACT) and then the
head matmuls with the activations as stationary operands, producing
token-major logits in PSUM. The DVE adds the output bias while
downcasting to fp16, and fully contiguous DMAs write the fp16 logits
back (host upcasts to fp32).

Scheduling, driven by trace analysis: the kernel is PE-bound mid-run
(~100% tensor busy), so all slack lives at the edges. The first ~25us
are DMA-limited (3 dynamic queues at ~60-80 GB/s each while ~4MB of
weights+x stream in), so the first blocks' heads — the only consumers
of wh — are deferred 2-3 blocks and the PE fills in with MLP-only
work; a short dummy-matmul warmup keeps the HAM clock ramp moving
while the first chunks land, and the first two blocks run k-outer so
matmuls stream with chunk arrivals. Output tiles are written in two
column halves on the two hardware DMA queues to shorten the drain
tail; x prefetch runs one block ahead on the software (gpsimd) queue.
"""
import sys

if "/opt/trn_rl_repo" not in sys.path:
    sys.path.append("/opt/trn_rl_repo")

import numpy as np

import concourse.bass as bass
from concourse import bacc
import concourse.mybir as mybir
import concourse.tile as tile
from concourse.bass import ts
from concourse.bass_utils import run_bass_kernel_spmd

F32 = mybir.dt.float32
F16 = mybir.dt.float16
RELU = mybir.ActivationFunctionType.Relu

# problem dims (hardcoded; kernel.py must be self-contained)
S, B, C = 3072, 32, 512
TS_ = S // 3                    # 1024 timesteps
NUM_SHAPES, NUM_COLORS, N_COMBINED = 64, 16, 920
NBRICK = NUM_SHAPES + NUM_COLORS  # 80
A = NBRICK + N_COMBINED           # 1000
NCORES = 8
BL = B // NCORES                  # 4 batch entries per core
NTOKC = TS_ * BL                  # 4096 tokens per name per core
TT = 128                          # tokens per tok-tile
TPB = TT // BL                    # 32 timesteps per tok-tile
KC = C // 128                     # 4 contraction chunks

_BUILD_CACHE = {}


def _build():
    if "nc" in _BUILD_CACHE:
        return _BUILD_CACHE["nc"]
    nc = bacc.Bacc("TRN2", target_bir_lowering=False, debug=False)

    # feature-major fp16 x: rows (ni*KC + k)*128 + p  <->  name ni, feature
    # k*128+p; cols are tokens t*BL+b
    xt_d = nc.declare_dram_parameter("xt", [2 * KC * 128, NTOKC], F16, isOutput=False)
    w1_d = nc.declare_dram_parameter("w1", [C, C], F16, isOutput=False)
    w2_d = nc.declare_dram_parameter("w2", [C, C], F16, isOutput=False)
    wh_d = nc.declare_dram_parameter("wh", [C, N_COMBINED], F16, isOutput=False)
    wsc_d = nc.declare_dram_parameter("wsc", [C, NBRICK], F16, isOutput=False)
    b12_d = nc.declare_dram_parameter("b12", [128, 2 * KC], F32, isOutput=False)
    bA_d = nc.declare_dram_parameter("biasA", [128, A], F32, isOutput=False)
    out_d = nc.declare_dram_parameter("out", [TS_ * BL, A], F16, isOutput=True)

    with tile.TileContext(nc) as tc:
        with (
            tc.tile_pool(name="const", bufs=1) as const,
            tc.tile_pool(name="xt", bufs=4) as xt_p,
            tc.tile_pool(name="h", bufs=4) as h_p,
            tc.tile_pool(name="osb", bufs=4) as o_p,
            tc.tile_pool(name="psh", bufs=4, space=bass.MemorySpace.PSUM) as ps_h,
            tc.tile_pool(name="psc", bufs=2, space=bass.MemorySpace.PSUM) as ps_c,
        ):
            # HAM warmup: a short burst of dummy matmuls at t=0 (on a memset
            # scratch, no DMA dependency) so the PE clock ramp is already in
            # progress while the first weight/x DMAs land. Kept minimal —
            # real matmuls during the ramp make progress, dummies don't.
            warm_src = const.tile([128, 128], F16, tag="warm")
            nc.vector.memset(warm_src[:], 0.0)
            warm = ps_h.tile([128, 512], F32, tag="hps")
            for _ in range(16):
                nc.tensor.matmul(warm[0:64, 0:64], warm_src[:, 0:64], warm_src[:, 0:64])
            # pre-fire the one-time ACT activation-table load so the first
            # real relu doesn't pay ~1.3us for it
            warm_act = const.tile([128, 1], F32, tag="warmact")
            nc.scalar.activation(warm_act[0:1, 0:1], warm_src[0:1, 0:1], RELU)

            # Startup DMA placement (per-HW-queue ~115 GB/s for 1KB-line
            # tiles; tiny-line tensors are descriptor-bound; SW-DGE gpsimd
            # queue coalesces into 4KB packets). Per-k-chunk tiles so the
            # k-outer block-0 compute streams with chunk arrivals (the tile
            # dependency is whole-DMA completion):
            #   sync:   w1 k0..k3, w2 k0..k3, biasA, then out-lo halves
            #   scalar: block-0 comb x, wh, then out-hi halves
            #   gpsimd: b1/b2 (tiny), wsc, block-0 brick x, then per-block
            #           x prefetch only
            w1_sb = []
            w2_sb = []
            for name, dram, out_list in (("w1", w1_d, w1_sb), ("w2", w2_d, w2_sb)):
                for k in range(KC):
                    t = const.tile([128, C], F16, tag=f"{name}_{k}")
                    out_list.append(t)
            for wsb, dram in ((w1_sb, w1_d), (w2_sb, w2_d)):
                for k in range(KC):
                    nc.sync.dma_start(wsb[k][:], dram[ts(k, 128), :])
            b12_sb = const.tile([128, 2 * KC], F32, tag="b12")
            nc.gpsimd.dma_start(b12_sb[:], b12_d[:, :])
            wh_sb = []
            wsc_sb = []
            for k in range(KC):
                t = const.tile([128, NBRICK], F16, tag=f"wsc_{k}")
                nc.gpsimd.dma_start(t[:], wsc_d[ts(k, 128), :])
                wsc_sb.append(t)

            def w1s(k, m):
                return w1_sb[k][:, ts(m, 128)]

            def w2s(k, m):
                return w2_sb[k][:, ts(m, 128)]

            def load_late_consts():
                # wh on scalar behind block-0 comb x; biasA last on sync
                for k in range(KC):
                    t = const.tile([128, N_COMBINED], F16, tag=f"wh_{k}")
                    nc.scalar.dma_start(t[:], wh_d[ts(k, 128), :])
                    wh_sb.append(t)
                bA_sb = const.tile([128, A], F32, tag="biasA")
                nc.sync.dma_start(bA_sb[:], bA_d[:, :])
                return bA_sb

            # heads for block i, emitted during block i+1: token-major
            # logits straight into PSUM ([0:80] brick, [80:1000] comb),
            # DVE bias-add + fp16 downcast, contiguous DMA out.
            def finals(pb):
                for t in range(pb["nt"]):
                    pco = ps_c.tile([128, 1024], F32, tag="combo")
                    for k in range(KC):
                        lhs = pb["h2"][k][:, ts(t, 128)]
                        nc.tensor.matmul(
                            pco[:, NBRICK:512],
                            lhs,
                            wh_sb[k][:, 0 : 512 - NBRICK],
                            start=(k == 0),
                            stop=(k == KC - 1),
                        )
                        nc.tensor.matmul(
                            pco[:, 512:A],
                            lhs,
                            wh_sb[k][:, 512 - NBRICK : N_COMBINED],
                            start=(k == 0),
                            stop=(k == KC - 1),
                        )
                    for k in range(KC):
                        nc.tensor.matmul(
                            pco[:, 0:NBRICK],
                            pb["xtb"][k][:, ts(t, 128)],
                            wsc_sb[k][:],
                            start=(k == 0),
                            stop=(k == KC - 1),
                        )
                    # bias-add + fp16 downcast in two column halves so the
                    # first half's DMA fires while the DVE does the second
                    # half; halves go to different queues (the last block's
                    # upper half uses the by-then-idle scalar HW queue so
                    # the drain tail is short)
                    ot = o_p.tile([128, A], F16, tag="osb")
                    r0 = (pb["t0"] + t * TPB) * BL
                    nc.vector.tensor_add(ot[:, 0:512], pco[:, 0:512], bA_sb[:, 0:512])
                    nc.sync.dma_start(out_d[r0 : r0 + TT, 0:512], ot[:, 0:512])
                    nc.vector.tensor_add(ot[:, 512:A], pco[:, 512:A], bA_sb[:, 512:A])
                    nc.scalar.dma_start(out_d[r0 : r0 + TT, 512:A], ot[:, 512:A])

            # ---- main loop over blocks of tok-tiles (128 tokens each) ----
            # ramp-up/ramp-down schedule: small blocks first (fast pipeline
            # fill, early HAM release) and last (short drain tail).
            sched = [4] * 7 + [2, 1, 1]
            assert sum(sched) == TS_ // TPB
            offs = [sum(sched[:i]) for i in range(len(sched))]
            # heads deferral: the first ~25us are DMA-limited (weights still
            # streaming in at ~60-80 GB/s per queue), so the first blocks'
            # heads — which need wh — are pushed back 1-2 extra blocks; the
            # PE fills that time with MLP work that only needs w1/w2/x. The
            # backlog is worked off two-per-block once wh has landed.
            nblk = len(sched)
            flush_n = {2: 1, 3: 2}
            flush_n.update({i: 1 for i in range(4, nblk)})

            # feature-major fp16 x loads, issued one block AHEAD so the
            # queue has a full block period to deliver. Block 0's comb x
            # goes on the scalar HW queue (parallel with w1 on sync) and
            # its brick x on gpsimd; later blocks use the gpsimd
            # software-DGE queue entirely.
            def issue_name_loads(bi, ni, eng):
                nt = sched[bi]
                tok0 = offs[bi] * TT
                W_ = nt * TT
                out_list = []
                for k in range(KC):
                    tl = xt_p.tile([128, W_], F16, tag=f"xt{ni}{k}")
                    r0 = (ni * KC + k) * 128
                    eng.dma_start(tl[:], xt_d[r0 : r0 + 128, tok0 : tok0 + W_])
                    out_list.append(tl)
                return out_list

            def issue_loads(bi):
                return (
                    issue_name_loads(bi, 1, nc.gpsimd),
                    issue_name_loads(bi, 0, nc.gpsimd),
                )

            xtc0 = issue_name_loads(0, 1, nc.scalar)
            bA_sb = load_late_consts()
            issued = {1: issue_loads(1)}
            # block 0's brick x goes behind block 1's x on the SW queue —
            # it is not read until the deferred heads flush
            issued[0] = (xtc0, issue_name_loads(0, 0, nc.gpsimd))
            pendings = []
            for bi, nt in enumerate(sched):
                ti0 = offs[bi]
                t0 = ti0 * TPB
                W_ = nt * TT  # tokens per name in this block
                xtc, xtb = issued.pop(bi)
                if bi + 1 < len(sched) and bi + 1 not in issued:
                    issued[bi + 1] = issue_loads(bi + 1)

                # deferred heads of earlier blocks (see note above)
                for _ in range(flush_n.get(bi, 0)):
                    finals(pendings.pop(0))

                # comb MLP: hT[m] = relu(W[:,m-chunk].T @ inT + b). Block 0
                # runs k-outer so matmuls stream with the arriving weight/x
                # DMA chunks; later blocks run m-outer so the ACT engine
                # drains each m-psum while the next accumulates.
                def layer(ws, inp, btag, hname):
                    hs_list = []
                    if bi < 2:
                        phs = []
                        for _m in range(KC):
                            ph = ps_h.tile([128, W_], F32, tag="hps")
                            phs.append(ph)
                        for k in range(KC):
                            for m in range(KC):
                                nc.tensor.matmul(
                                    phs[m][:],
                                    ws(k, m),
                                    inp[k][:],
                                    start=(k == 0),
                                    stop=(k == KC - 1),
                                )
                        for m in range(KC):
                            hs = h_p.tile([128, W_], F16, tag=f"{hname}_{m}")
                            nc.scalar.activation(
                                hs[:],
                                phs[m][:],
                                RELU,
                                bias=b12_sb[:, btag + m : btag + m + 1],
                                scale=1.0,
                            )
                            hs_list.append(hs)
                    else:
                        for m in range(KC):
                            ph = ps_h.tile([128, W_], F32, tag="hps")
                            for k in range(KC):
                                nc.tensor.matmul(
                                    ph[:],
                                    ws(k, m),
                                    inp[k][:],
                                    start=(k == 0),
                                    stop=(k == KC - 1),
                                )
                            hs = h_p.tile([128, W_], F16, tag=f"{hname}_{m}")
                            nc.scalar.activation(
                                hs[:],
                                ph[:],
                                RELU,
                                bias=b12_sb[:, btag + m : btag + m + 1],
                                scale=1.0,
                            )
                            hs_list.append(hs)
                    return hs_list

                h1 = layer(w1s, xtc, 0, "h1")
                h2 = layer(w2s, h1, KC, "h2")

                pendings.append({"h2": h2, "xtb": xtb, "t0": t0, "nt": nt})
            assert len(pendings) == 1
            pendings[0]["last"] = True
            finals(pendings[0])

    nc.compile()
    _BUILD_CACHE["nc"] = nc
    return nc


def _prepare_inputs(inputs):
    """Host-side prep: validate/normalize routing, shard over batch,
    pre-pack x feature-major fp16, replicate weights. Returns in_maps
    for the 8 cores."""
    x = np.ascontiguousarray(np.asarray(inputs["x"], dtype=np.float32))
    readout_x = np.asarray(inputs["readout_x"], dtype=np.int32)
    W1 = np.asarray(inputs["W1"], dtype=np.float32)
    W2 = np.asarray(inputs["W2"], dtype=np.float32)
    Wh = np.asarray(inputs["Wh"], dtype=np.float32)
    Ws = np.asarray(inputs["Ws"], dtype=np.float32)
    Wc = np.asarray(inputs["Wc"], dtype=np.float32)
    b1 = np.asarray(inputs["b1"], dtype=np.float32)
    b2 = np.asarray(inputs["b2"], dtype=np.float32)
    bh = np.asarray(inputs["bh"], dtype=np.float32)
    bs = np.asarray(inputs["bs"], dtype=np.float32)
    bc = np.asarray(inputs["bc"], dtype=np.float32)

    # The kernel hardcodes the cyclic PAD/brick/comb routing. If the actual
    # readout pattern differs, permute x on the host so the device sees the
    # canonical layout (mirrors jnp.nonzero(..., size=ntok) semantics).
    ntok = TS_ * B
    rf = readout_x.reshape(-1)
    canonical = np.array_equal(
        readout_x, np.broadcast_to((np.arange(S, dtype=np.int32) % 3)[:, None], (S, B))
    )
    if not canonical:
        xf = x.reshape(S * B, C)
        xc = np.zeros_like(x).reshape(S * B, C)
        for name_idx in (1, 2):
            idx = np.nonzero(rf == name_idx)[0]
            if idx.shape[0] < ntok:
                idx = np.pad(idx, (0, ntok - idx.shape[0]))
            else:
                idx = idx[:ntok]
            tgt = (3 * (np.arange(ntok) // B) + name_idx) * B + (np.arange(ntok) % B)
            xc[tgt] = xf[idx]
        x = xc.reshape(S, B, C)

    # feature-major fp16 pack: [name, core, C, t, b] with brick first
    xh = x.astype(np.float16).reshape(TS_, 3, NCORES, BL, C)
    xp = xh[:, 1:3].transpose(1, 2, 4, 0, 3)  # [name, core, C, t, b]

    Wsc = np.ascontiguousarray(np.concatenate([Ws, Wc], axis=1).astype(np.float16))
    W1h = np.ascontiguousarray(W1.astype(np.float16))
    W2h = np.ascontiguousarray(W2.astype(np.float16))
    Whh = np.ascontiguousarray(Wh.astype(np.float16))
    b12 = np.ascontiguousarray(
        np.concatenate([b1.reshape(KC, 128).T, b2.reshape(KC, 128).T], axis=1)
    )
    biasA = np.concatenate([bs, bc, bh])
    biasA_b = np.ascontiguousarray(np.broadcast_to(biasA, (128, A)))

    in_maps = []
    for c in range(NCORES):
        # rows: (ni*KC + k)*128 + p = ni*512 + feature, brick ni=0 from
        # readout name 1, comb ni=1 from name 2
        xt_core = np.ascontiguousarray(
            xp[:, c].reshape(2 * C, NTOKC)
        )
        in_maps.append(
            {
                "xt": xt_core,
                "w1": W1h,
                "w2": W2h,
                "wh": Whh,
                "wsc": Wsc,
                "b12": b12,
                "biasA": biasA_b,
            }
        )
    return in_maps


def _run(inputs, trace=False, trace_kwargs=None):
    nc = _build()
    in_maps = _prepare_inputs(inputs)
    res = run_bass_kernel_spmd(
        nc,
        in_maps,
        list(range(NCORES)),
        trace=trace,
        **(trace_kwargs or {}),
    )
    out = np.empty((TS_, B, A), dtype=np.float32)
    for c in range(NCORES):
        out[:, c * BL : (c + 1) * BL, :] = (
            res.results[c]["out"].reshape(TS_, BL, A).astype(np.float32)
        )
    return out, res


def kernel(**inputs) -> np.ndarray:
    out, _ = _run(inputs, trace=False)
    return out


if __name__ == "__main__":
    nc = _build()
    print("built OK")
